# revision 34
# baseline (speedup 1.0000x reference)
"""Trainium2 Bass kernel for nn_BasisJastrow.

Math (per batch element b):
    J_b = (1/P) * sum_{i<j} chi_j^T C chi_i ,   P = N(N-1)/2, C = coeff.reshape(Nb, Nb)

Device decomposition (per core; data-parallel over the batch axis).
The 48 per-core batches are split into two halves on disjoint SBUF partition
ranges (A = batches 0..23 on partitions 0:64, B = batches 24..47 on 64:128);
every PE matmul runs as a concurrent pair on disjoint systolic quadrants
(0,0)/(64,64).  All matmul data is bf16 (single PE pass, half the DMA bytes);
accumulation stays fp32 in PSUM.

  layout  Xl[n + 64*half, (b,u)]   n=64 on partitions, 24 batches * 32 per half
  x is DMAed in 4 column-chunks split across both HWDGE rings (SP + ACT) so
  the first chunk lands earlier and the rings run in parallel.

  phase 1 S_c  = Lt.T @ X_c        exclusive prefix sums over particles (PE)
  cast    s_sb = bf16(S_c)         PSUM->SBUF on the ACT engine (the ACT
                                   activation table is preloaded by a dummy
                                   memzero inside the DMA window)
  phase 2 Q_p  = Xp.T @ Sp         2-batch cross-Gram [64,64] per half; diag
                                   32x32 blocks are G_b, off-diag garbage;
                                   one pe increment per pair
  phase 3 r[:,p] = sum_f Q_p*CD2   ONE fused DVE scalar_tensor_tensor per
                                   pair ((q*1)*cd2 with accum_out, a single
                                   pass; CD2 = blockdiag(C,C)/P masks the
                                   off-diag). cast0 runs on DVE (no ACT
                                   table); casts 1-3 on ACT.
  out     r_sb [128,12] fp32 shipped in two DMAs (cols 0:8 mid-conveyor,
          8:12 at the end); the tiny 32-partition-group sum
          (128*12 -> 4*12) happens on the host during unshard.

Raw Bass (explicit engine blocks + semaphores): the walrus build in this
container rejects any instruction carrying more than one sync wait, which
rules out Tile's generated sem placement; raw Bass emits waits standalone.
"""

import sys

for _p in ("/opt/trn_rl_repo",):
    if _p not in sys.path:
        sys.path.insert(0, _p)

import numpy as np

import concourse.bass as bass
from concourse import mybir
from concourse.bass_utils import run_bass_kernel_spmd

B, N, Nb = 384, 64, 32
NCORES = 8
BS = B // NCORES            # 48 batches per core
HB = BS // 2                # 24 batches per half
NP = HB // 2                # 12 concurrent gram pairs
NPAIR = N * (N - 1) // 2    # 2016
F32 = mybir.dt.float32
BF16 = mybir.dt.bfloat16
MM_DT = BF16

FREE = HB * Nb              # 768 free columns per half
NCHUNK = 4
CHUNK_PAIRS = [2, 4, 3, 3]
CHUNK_OFF = [0, 128, 384, 576, 768]
PAIR_CHUNK = [0, 0, 1, 1, 1, 1, 2, 2, 2, 3, 3, 3]
N_WARM = 3                  # PE warmup matmuls during the input DMA window
NQB = 4                     # gram psum banks
BANK_PAIRS = [2, 4, 3, 3]   # pairs per bank; small first bank -> DVE starts early
BANK_START = [0, 2, 6, 9]


def build_nc() -> bass.Bass:
    nc = bass.Bass()
    # 8 queues per DMA ring instead of 16: halves the flat per-dma_start
    # descriptor-write time (~700ns -> ~350ns) at both head and tail; ring
    # bandwidth stays ample for these transfer sizes.
    for _q in nc.m.queues:
        _q.num_queues = 8

    x_d = nc.dram_tensor("x", [128, FREE], MM_DT, kind="ExternalInput")
    cp_d = nc.dram_tensor("cp", [128, 64], F32, kind="ExternalInput")
    r_d = nc.dram_tensor("r", [128, NP], F32, kind="ExternalOutput")

    from contextlib import ExitStack

    with ExitStack() as ctx:
        x_sb = ctx.enter_context(nc.sbuf_tensor("x_sb", [128, FREE], MM_DT))
        s_sb = ctx.enter_context(nc.sbuf_tensor("s_sb", [128, FREE], MM_DT))
        cp_sb = ctx.enter_context(nc.sbuf_tensor("cp_sb", [128, 64], F32))
        w_sb = ctx.enter_context(nc.sbuf_tensor("w_sb", [128, 84], F32))
        lt_sb = ctx.enter_context(nc.sbuf_tensor("lt_sb", [128, N], MM_DT))
        w2_sb = ctx.enter_context(nc.sbuf_tensor("w2_sb", [1, 1], F32))
        e_sb = ctx.enter_context(nc.sbuf_tensor("e_sb", [128, NP, 64], F32))
        r_sb = ctx.enter_context(nc.sbuf_tensor("r_sb", [128, NP], F32))
        s_ps = [
            ctx.enter_context(
                nc.psum_tensor(
                    f"s_ps{c}", [128, CHUNK_OFF[c + 1] - CHUNK_OFF[c]], F32
                )
            )
            for c in range(NCHUNK)
        ]
        q_ps = [
            ctx.enter_context(
                nc.psum_tensor(f"q_ps{k}", [128, BANK_PAIRS[k] * 64], F32)
            )
            for k in range(NQB)
        ]
        # warmup scratch aliases q_ps[3]'s bank (P2 fully rewrites it later)
        warm_ps = q_ps[3]
        dma_x = [
            ctx.enter_context(nc.semaphore(f"dma_x{c}")) for c in range(NCHUNK)
        ]
        dma_c = ctx.enter_context(nc.semaphore("dma_c"))
        dma_o = ctx.enter_context(nc.semaphore("dma_o"))
        pe = ctx.enter_context(nc.semaphore("pe"))
        act = ctx.enter_context(nc.semaphore("act"))
        dve = ctx.enter_context(nc.semaphore("dve"))
        gp_w = ctx.enter_context(nc.semaphore("gp_w"))
        gp_r = ctx.enter_context(nc.semaphore("gp_r"))
        block = ctx.enter_context(nc.Block(no_gpsimd_drain=True))
        lt = lt_sb[:]
        # pe ledger: P1c0..c3 = 1..4, P2 pair p -> 5+p (p=0..11)
        # dve ledger: mul0=1 mul1=2 red0=3 mul2=4 red1=5 mul3=6 red2=7 red3=8
        MUL_DONE = [1, 2, 4, 6]

        def bank_of(p):
            for k in range(NQB):
                if p < BANK_START[k] + BANK_PAIRS[k]:
                    return k
            raise AssertionError

        @block.sync
        def _(sync):
            for c in (0, 2):
                cs = slice(CHUNK_OFF[c], CHUNK_OFF[c + 1])
                sync.dma_start(out=x_sb[:, cs], in_=x_d[:, cs]).then_inc(
                    dma_x[c], 16
                )
            sync.wait_ge(dve, 9)
            sync.dma_start(out=r_d[:, 0:8], in_=r_sb[:, 0:8]).then_inc(
                dma_o, 16
            )
            sync.wait_ge(dve, 13)
            sync.dma_start(out=r_d[:, 8:NP], in_=r_sb[:, 8:NP]).then_inc(
                dma_o, 16
            )

        @block.scalar
        def _(scalar):
            for c in (1, 3):
                cs = slice(CHUNK_OFF[c], CHUNK_OFF[c + 1])
                scalar.dma_start(out=x_sb[:, cs], in_=x_d[:, cs]).then_inc(
                    dma_x[c], 16
                )
            # dummy ACTIVATE so walrus's act-table load runs in the DMA window
            scalar.memzero(w2_sb[:])
            CAST_PE = {1: 2, 2: 5, 3: 6}
            for c in range(1, NCHUNK):
                cs = slice(CHUNK_OFF[c], CHUNK_OFF[c + 1])
                scalar.wait_ge(pe, CAST_PE[c])
                scalar.copy(out=s_sb[:, cs], in_=s_ps[c][:]).then_inc(act, 1)

        @block.gpsimd
        def _(gpsimd):
            gpsimd.dma_start(out=cp_sb[:], in_=cp_d[:]).then_inc(dma_c, 16)
            gpsimd.memset(w_sb[:], 1.0).then_inc(gp_w, 1)
            gpsimd.wait_ge(gp_w, 1)
            for h in range(2):
                hs = slice(h * 64, (h + 1) * 64)
                gpsimd.affine_select(
                    out=lt_sb[hs, :],
                    in_=w_sb[hs, 0:N],
                    pattern=[[1, N]],
                    compare_op=mybir.AluOpType.is_gt,
                    fill=0.0,
                    base=0,
                    channel_multiplier=-1,
                ).then_inc(gp_w, 1)


        @block.tensor
        def _(tensor):
            # HAM warmup on memset scratch while the input DMAs land
            tensor.wait_ge(gp_w, 1)
            for w in range(N_WARM):
                tensor.matmul(
                    warm_ps[0:4, w * 64 : (w + 1) * 64],
                    w_sb[0:64, 0:4],
                    w_sb[0:64, 4:68],
                    start=True,
                    stop=True,
                    tile_position=(0, 0),
                )
            tensor.wait_ge(gp_w, 3)

            def p1(c):
                cs = slice(CHUNK_OFF[c], CHUNK_OFF[c + 1])
                tensor.wait_ge(dma_x[c], 16)
                tensor.matmul(
                    s_ps[c][0:64, :],
                    lt[0:64, :],
                    x_sb[0:64, cs],
                    start=True,
                    stop=True,
                    tile_position=(0, 0),
                )
                tensor.matmul(
                    s_ps[c][64:128, :],
                    lt[64:128, :],
                    x_sb[64:128, cs],
                    start=True,
                    stop=True,
                    tile_position=(64, 64),
                ).then_inc(pe, 1)

            def p2(p):
                ps_ = slice(p * 64, (p + 1) * 64)
                bk = bank_of(p)
                qi = p - BANK_START[bk]
                q = q_ps[bk][:, qi * 64 : (qi + 1) * 64]
                tensor.matmul(
                    q[0:64, :],
                    x_sb[0:64, ps_],
                    s_sb[0:64, ps_],
                    start=True,
                    stop=True,
                    tile_position=(0, 0),
                )
                tensor.matmul(
                    q[64:128, :],
                    x_sb[64:128, ps_],
                    s_sb[64:128, ps_],
                    start=True,
                    stop=True,
                    tile_position=(64, 64),
                ).then_inc(pe, 1)

            p1(0)
            p1(1)
            tensor.wait_ge(dve, 1)
            p2(0)
            p2(1)
            p1(2)
            p1(3)
            tensor.wait_ge(act, 1)
            p2(2)
            p2(3)
            p2(4)
            p2(5)
            tensor.wait_ge(act, 2)
            p2(6)
            p2(7)
            p2(8)
            tensor.wait_ge(act, 3)
            p2(9)
            p2(10)
            p2(11)


        @block.vector
        def _(vector):
            cd2 = cp_sb[:]
            cd4 = bass.AP(
                tensor=cd2.tensor,
                offset=cd2.offset,
                ap=[list(cd2.ap[0]), [0, max(BANK_PAIRS)], list(cd2.ap[1])],
            )

            def mul(k):
                if k == 0:
                    vector.wait_ge(dma_c, 16)
                npb = BANK_PAIRS[k]
                vector.wait_ge(pe, 4 + BANK_START[k] + npb)
                q3 = q_ps[k][:].rearrange("p (r f) -> p r f", r=npb)
                e3 = e_sb[:, BANK_START[k] : BANK_START[k] + npb, :]
                vector.tensor_tensor(
                    out=e3, in0=q3, in1=cd4[:, 0:npb, :], op=mybir.AluOpType.mult
                ).then_inc(dve, 1)

            def red(k):
                vector.wait_ge(dve, MUL_DONE[k])
                npb = BANK_PAIRS[k]
                e3 = e_sb[:, BANK_START[k] : BANK_START[k] + npb, :]
                vector.tensor_reduce(
                    out=r_sb[:, BANK_START[k] : BANK_START[k] + npb],
                    in_=e3,
                    axis=mybir.AxisListType.X,
                    op=mybir.AluOpType.add,
                ).then_inc(dve, 1)

            vector.wait_ge(pe, 1)
            vector.tensor_copy(
                s_sb[:, CHUNK_OFF[0] : CHUNK_OFF[1]], s_ps[0][:]
            ).then_inc(dve, 1)
            FUSED_STT = True
            if FUSED_STT:
                # fused (q*1.0)*cd2 with accum -> r per pair
                PAIR_PE = [3, 4, 7, 8, 9, 10, 11, 12, 13, 14, 15, 16]
                for p in range(NP):
                    if p == 0:
                        vector.wait_ge(dma_c, 16)
                    bk = bank_of(p)
                    qi = p - BANK_START[bk]
                    q = q_ps[bk][:, qi * 64 : (qi + 1) * 64]
                    vector.wait_ge(pe, PAIR_PE[p])
                    vector.scalar_tensor_tensor(
                        out=e_sb[:, p, :],
                        in0=q,
                        scalar=1.0,
                        in1=cd2,
                        op0=mybir.AluOpType.mult,
                        op1=mybir.AluOpType.mult,
                        accum_out=r_sb[:, p : p + 1],
                    ).then_inc(dve, 1)
            else:
                mul(0)
                mul(1)
                red(0)
                mul(2)
                red(1)
                mul(3)
                red(2)
                red(3)

    return nc


def make_consts(jastrow_coeff: np.ndarray):
    C = np.asarray(jastrow_coeff, dtype=np.float32).reshape(Nb, Nb)
    bd2 = np.zeros((64, 64), dtype=np.float32)
    for i in range(2):
        bd2[32 * i : 32 * (i + 1), 32 * i : 32 * (i + 1)] = C / NPAIR
    cp = np.zeros((128, 64), dtype=np.float32)
    cp[0:64] = bd2
    cp[64:128] = bd2
    return cp


def shard_x(basis_single_body: np.ndarray):
    import ml_dtypes

    x = np.asarray(basis_single_body, dtype=np.float32)
    xt = np.ascontiguousarray(x.transpose(1, 0, 2))  # [N, B, Nb]
    out = []
    for m in range(NCORES):
        sl = xt[:, m * BS : (m + 1) * BS, :]
        a = sl[:, 0:HB, :].reshape(N, FREE)
        b = sl[:, HB:BS, :].reshape(N, FREE)
        out.append(
            np.ascontiguousarray(np.concatenate([a, b], axis=0)).astype(
                ml_dtypes.bfloat16
            )
        )
    return out


def unpack_j(r: np.ndarray) -> np.ndarray:
    """r[128, p] -> per-core J[48]: group g = p//32 of partitions sums to
    batch 24*(g//2) + 2p + (g%2)."""
    r = np.asarray(r, dtype=np.float32).reshape(4, 32, NP)
    j = r.sum(axis=1)  # [4, NP]
    out = np.empty(BS, dtype=np.float32)
    for g in range(4):
        for p in range(NP):
            out[24 * (g // 2) + 2 * p + (g % 2)] = j[g, p]
    return out


_NC_CACHE: list = []


def kernel(basis_single_body: np.ndarray, jastrow_coeff: np.ndarray) -> np.ndarray:
    if not _NC_CACHE:
        _NC_CACHE.append(build_nc())
    nc = _NC_CACHE[0]

    cp = make_consts(jastrow_coeff)
    shards = shard_x(basis_single_body)
    in_maps = [{"x": s, "cp": cp} for s in shards]

    res = run_bass_kernel_spmd(nc, in_maps, core_ids=list(range(NCORES)))
    return np.concatenate([unpack_j(np.asarray(r["r"])) for r in res.results])
